# revision 20
# baseline (speedup 1.0000x reference)
"""Trainium2 Bass kernel for BindingAffinityModel (2x 3-layer GCN + MLP).

Strategy (8 NeuronCores, one SPMD program with per-core If branches):
  - dst-range shard nodes/edges per core; each core gathers neighbor rows
    from a replicated full node table in HBM via SWDGE dma_gather.
  - segment-sum via one-hot matmuls on TensorE accumulating in PSUM
    (per-element has_written semantics handle overlapping column slices).
  - GCN algebra reordering: aggregate in input space (Ahat@x)@W', with bn
    folded into W'/b', and D^-1/2 applied as a table pre-scale (src side)
    plus a per-column post-scale (dst side). Self loops are (d,d) slots.
  - between layers: AllGather of the per-core [Nc,64] shard into the full
    table; final mean-pool partials AllReduce'd; small MLP replicated.
All graph-dependent structure (edges, degrees, batch segments) is
preprocessed on host into index/metadata arrays; per-core instruction
streams are fully specialized inside If(partition_id==c) blocks.

Perf: gathers dominate device time. They run on 4 SWDGE queues
(round-robin per call; each queue ~ one DMA engine's bus share), with
edges sorted by src within each 128-slot block so descriptors walk
ascending HBM addresses (the dst one-hot absorbs the permutation).
Protein/mol phases are interleaved (p1,AGp,m1,AGm,p2,...) so each side's
AllGather and all PE/DVE work hide under the other side's gathers.
single_packet must stay False: True wedges the device.
"""
import sys
import numpy as np

sys.path.insert(0, "/opt/trn_rl_repo")

P = 128
SPAN = 48
WCHUNK = 16
BN_EPS = 1e-5
NCORES = 8
G = 256
SCRATCH = 40  # extra pool columns for multi-piece graphs

_wsplit_ctr = [0]


def _split_multi_waits(nc):
    """Workaround: this walrus build rejects any instruction carrying >1
    sync wait ("Too many sync wait commands"). Move extra waits onto NoOp
    carriers inserted just before the instruction on the same engine."""
    import concourse.mybir as mybir
    nsplit = 0
    for f in nc.m.functions:
        for bb in f.blocks:
            il = bb.instructions
            i = 0
            while i < len(il):
                inst = il[i]
                si = inst.sync_info
                if si is not None and si.on_wait and len(si.on_wait) > 1:
                    import os as _os
                    if _os.environ.get("KSPLITLOG"):
                        print("SPLIT:", inst.name, type(inst).__name__,
                              inst.engine, len(si.on_wait),
                              [w.ant_name for w in si.on_wait])
                    waits = list(si.on_wait)
                    si.on_wait = waits[-1:]
                    for w in waits[:-1]:
                        _wsplit_ctr[0] += 1
                        nop = mybir.InstNoOp(
                            name=f"Wsplit-{_wsplit_ctr[0]}",
                            engine=inst.engine, ins=[], outs=[])
                        nop.sync_info = mybir.SyncInfo(on_wait=[w], on_update=[])
                        il.insert(i, nop)
                        i += 1
                        nsplit += 1
                i += 1
    return nsplit


# ---------------------------------------------------------------- host prep

def _fold_bn(w, b, g, bb, m, v):
    s = (np.asarray(g, np.float64) / np.sqrt(np.asarray(v, np.float64) + BN_EPS))
    wf = (np.asarray(w, np.float64) * s[None, :]).astype(np.float32)
    bf = ((np.asarray(b, np.float64) - np.asarray(m, np.float64)) * s
          + np.asarray(bb, np.float64)).astype(np.float32)
    return wf, bf


def _prep_side(x, edge, batch, N, Fin, WIN, RANGE_SZ, weights):
    """Preprocess one graph side. Returns config + per-core arrays/metadata."""
    src = np.asarray(edge[0], dtype=np.int64)
    dst = np.asarray(edge[1], dtype=np.int64)
    batch = np.asarray(batch, dtype=np.int64)
    x = np.asarray(x, dtype=np.float32)

    NCs = N // NCORES
    NW = NCs // WIN          # windows per core
    WPT = 1                  # windows per gather-tile
    NT = (NW + WPT - 1) // WPT
    R = (N + RANGE_SZ - 1) // RANGE_SZ

    deg = np.bincount(dst, minlength=N).astype(np.float64) + 1.0
    dinv = (1.0 / np.sqrt(deg)).astype(np.float32)

    tab1 = np.zeros((N, 64), dtype=np.float32)
    tab1[:, :Fin] = x * dinv[:, None]

    w1, b1 = _fold_bn(weights["w1"], weights["b1"], weights["bn1_g"],
                      weights["bn1_b"], weights["bn1_m"], weights["bn1_v"])
    w1p = np.zeros((64, 64), dtype=np.float32)
    w1p[:Fin] = w1
    w2, b2 = _fold_bn(weights["w2"], weights["b2"], weights["bn2_g"],
                      weights["bn2_b"], weights["bn2_m"], weights["bn2_v"])
    w3 = np.asarray(weights["w3"], dtype=np.float32)
    b3 = np.asarray(weights["b3"], dtype=np.float32)

    cores = []
    for c in range(NCORES):
        lo, hi = c * NCs, (c + 1) * NCs
        sel = (dst >= lo) & (dst < hi)
        es = np.concatenate([src[sel], np.arange(lo, hi, dtype=np.int64)])
        ed = np.concatenate([dst[sel], np.arange(lo, hi, dtype=np.int64)])
        dl = ed - lo
        wv = dl // WIN
        tv = wv // WPT
        rv = es // RANGE_SZ
        order = np.lexsort((es, dl, rv, tv))
        es, dl, rv, tv = es[order], dl[order], rv[order], tv[order]
        wv = dl // WIN

        # segment boundaries at (tile, range, window) change
        key = (tv * R + rv) * NW + wv
        cuts = np.flatnonzero(np.diff(key)) + 1
        seg_bounds = np.concatenate([[0], cuts, [len(key)]])

        idx_slots, dst_slots = [], []
        subs = []            # (tile, window, off, width)
        calls = []           # (tile, r, slot_out0, slot_out1)
        tiles_nsub = [0] * NT
        tiles_slots = [0] * NT
        cur_call = None
        out_pos = 0

        for si in range(len(seg_bounds) - 1):
            s0, s1 = int(seg_bounds[si]), int(seg_bounds[si + 1])
            t, r, w = int(tv[s0]), int(rv[s0]), int(wv[s0])
            if cur_call is None or cur_call[0] != t or cur_call[1] != r:
                if cur_call is not None:
                    calls.append((cur_call[0], cur_call[1], cur_call[2], out_pos))
                cur_call = (t, r, out_pos)
            seg_dl = dl[s0:s1]
            seg_es = es[s0:s1]
            win_base = w * WIN
            p = s0
            while p < s1:
                sub_base = int(dl[p])
                limit = sub_base + SPAN
                q = min(p + 128, s1)
                over = np.flatnonzero(seg_dl[p - s0:q - s0] >= limit)
                if len(over):
                    q = p + int(over[0])
                n = q - p
                idx_blk = np.zeros(128, dtype=np.int16)
                dst_blk = np.full(128, -1.0, dtype=np.float32)
                # ascending-src order within the block: gather descriptors
                # then walk increasing HBM addresses (dst one-hot handles
                # the permutation)
                bs = seg_es[p - s0:q - s0]
                bd = seg_dl[p - s0:q - s0]
                perm = np.argsort(bs, kind="stable")
                idx_blk[:n] = (bs[perm] - r * RANGE_SZ).astype(np.int16)
                dst_blk[:n] = (bd[perm] - sub_base).astype(np.float32)
                idx_slots.append(idx_blk)
                dst_slots.append(dst_blk)
                off = sub_base - win_base
                width = min(SPAN, WIN - off)
                assert off + width <= WIN and seg_dl[q - s0 - 1] - sub_base < width
                subs.append((t, w, off, width))
                tiles_nsub[t] += 1
                tiles_slots[t] += 128
                out_pos += 128
                p = q
        if cur_call is not None:
            calls.append((cur_call[0], cur_call[1], cur_call[2], out_pos))

        idx_arr = np.concatenate(idx_slots) if idx_slots else np.zeros(0, np.int16)
        dst_arr = (np.concatenate(dst_slots) if dst_slots
                   else np.zeros(0, np.float32))

        first_seen, last_idx = {}, {}
        for j, (t, w, off, width) in enumerate(subs):
            first_seen.setdefault((t, w), j)
            last_idx[(t, w)] = j
        starts = set(first_seen.values())
        stops = set(last_idx.values())

        # pooling pieces: (window, start_in_window, length, col, add_to_g)
        pieces = []
        scratch_used = 0
        counts_l = np.bincount(batch, minlength=G)
        cum = np.concatenate([[0], np.cumsum(counts_l)])
        for g in range(G):
            glo, ghi = max(int(cum[g]), lo), min(int(cum[g + 1]), hi)
            if glo >= ghi:
                continue
            first = True
            a = glo
            while a < ghi:
                w = (a - lo) // WIN
                b_ = min(ghi, lo + (w + 1) * WIN)
                if first:
                    col, addto = g, -1
                else:
                    col, addto = G + scratch_used, g
                    scratch_used += 1
                pieces.append((w, (a - lo) - w * WIN, b_ - a, col, addto))
                first = False
                a = b_
        assert scratch_used <= SCRATCH, scratch_used

        cores.append(dict(
            idx=idx_arr, dstrel=dst_arr, subs=subs, calls=calls,
            starts=starts, stops=stops, tiles_nsub=tiles_nsub,
            tiles_slots=tiles_slots, pieces=pieces,
            dinv_shard=dinv[lo:hi],
        ))

    S = max(len(cc["idx"]) for cc in cores)
    S = max(((S + 2047) // 2048) * 2048, 2048)
    NSUB = S // 128
    CTILE = max(max((s // 128 for s in cc["tiles_slots"]), default=1)
                for cc in cores)
    idx_w = np.zeros((NCORES, P, S // 16), dtype=np.int16)
    dst_w = np.full((NCORES, P, NSUB), -1.0, dtype=np.float32)
    dinv_rep = np.zeros((NCORES, 64, NCs), dtype=np.float32)
    for c, cc in enumerate(cores):
        n = len(cc["idx"])
        idx_pad = np.concatenate([cc["idx"], np.zeros(S - n, np.int16)])
        dst_pad = np.concatenate([cc["dstrel"], np.full(S - n, -1.0, np.float32)])
        idx_w[c] = np.tile(idx_pad.reshape(-1, 16).T, (8, 1))
        dst_w[c] = dst_pad.reshape(-1, 128).T
        dinv_rep[c] = np.tile(cc["dinv_shard"][None, :], (64, 1))

    counts = np.bincount(batch, minlength=G).astype(np.float32)
    cinv = (1.0 / np.maximum(counts, 1.0)).astype(np.float32)

    return dict(
        N=N, NCs=NCs, WIN=WIN, NW=NW, WPT=WPT, NT=NT, R=R, RANGE_SZ=RANGE_SZ,
        S=S, NSUB=NSUB, CTILE=CTILE,
        tab1=tab1, w=[w1p, w2, w3], b=[b1, b2, b3],
        idx_w=idx_w, dst_w=dst_w, dinv_rep=dinv_rep, cinv=cinv,
        cores=cores,
    )


# ---------------------------------------------------------------- builder

def _build(sides, f_b4):
    import concourse.bass as bass
    import concourse.mybir as mybir
    import concourse.tile as tile
    from concourse import library_config

    f32 = mybir.dt.float32
    i16 = mybir.dt.int16
    AOP = mybir.AluOpType
    AFT = mybir.ActivationFunctionType

    nc = bass.Bass("TRN2", num_devices=NCORES, num_swdge_queues=4)

    tens = {}

    def inp(name, shape, dtype=f32):
        tens[name] = nc.dram_tensor(name, list(shape), dtype,
                                    kind="ExternalInput")
        return tens[name]

    for sname, sd in sides.items():
        inp(f"{sname}_tab1", (sd["N"], 64))
        inp(f"{sname}_idx", (P, sd["S"] // 16), i16)
        inp(f"{sname}_dstrel", (P, sd["NSUB"]))
        inp(f"{sname}_dinv", (64, sd["NCs"]))
        for l in range(3):
            fo = sd["w"][l].shape[1]
            inp(f"{sname}_w{l}", (64, fo))
            inp(f"{sname}_b{l}", (fo, 1))
    inp("iota", (P, SPAN))
    inp("identity", (P, P))
    inp("cinv", (P, 2 * G))
    inp("f_w1", (256, 128)); inp("f_b1", (128, 1))
    inp("f_w2", (128, 64)); inp("f_b2", (64, 1))
    inp("f_w3", (64, 32)); inp("f_b3", (32, 1))
    inp("f_w4", (32, 1))
    out_t = nc.dram_tensor("out", [G, 1], f32, kind="ExternalOutput")

    for sname, sd in sides.items():
        tens[f"{sname}_ag"] = nc.dram_tensor(
            f"{sname}_ag", [sd["NCs"], 64], f32)
        tens[f"{sname}_tab2"] = nc.dram_tensor(
            f"{sname}_tab2", [sd["N"], 64], f32, addr_space="Shared")
        tens[f"{sname}_tab3"] = nc.dram_tensor(
            f"{sname}_tab3", [sd["N"], 64], f32, addr_space="Shared")
    tens["cc_in"] = nc.dram_tensor("cc_in", [P, 2 * G], f32)
    tens["cc_out"] = nc.dram_tensor("cc_out", [P, 2 * G], f32,
                                    addr_space="Shared")

    nc.gpsimd.load_library(library_config.attnmlp)

    rg = [list(range(NCORES))]

    with tile.TileContext(nc) as tc:
        pid = nc.partition_id()
        with tc.tile_pool(name="const", bufs=1) as cpool, \
             tc.tile_pool(name="sbuf", bufs=3) as pool, \
             tc.tile_pool(name="gpool", bufs=3) as gpool, \
             tc.tile_pool(name="wpool", bufs=3) as wpool, \
             tc.tile_pool(name="agg", bufs=4, space="PSUM") as psum_agg, \
             tc.tile_pool(name="py", bufs=2, space="PSUM") as psum_y, \
             tc.tile_pool(name="pt", bufs=2, space="PSUM") as psum_t:

            iota_t = cpool.tile([P, SPAN], f32)
            nc.sync.dma_start(out=iota_t[:], in_=tens["iota"][:])
            ident = cpool.tile([P, P], f32)
            nc.sync.dma_start(out=ident[:], in_=tens["identity"][:])
            zero_t = cpool.tile([P, 512], f32)
            nc.vector.memset(zero_t[:], 0.0)

            res = {}
            for sname, sd in sides.items():
                dst_sb = cpool.tile([P, sd["NSUB"]], f32, tag=f"{sname}_dst")
                nc.sync.dma_start(out=dst_sb[:], in_=tens[f"{sname}_dstrel"][:])
                ws, bs = [], []
                for l in range(3):
                    fo = sd["w"][l].shape[1]
                    wt = cpool.tile([64, fo], f32, tag=f"{sname}_w{l}")
                    nc.sync.dma_start(out=wt[:], in_=tens[f"{sname}_w{l}"][:])
                    bt = cpool.tile([fo, 1], f32, tag=f"{sname}_b{l}")
                    nc.sync.dma_start(out=bt[:], in_=tens[f"{sname}_b{l}"][:])
                    ws.append(wt)
                    bs.append(bt)
                pool_sb = cpool.tile([P, G + SCRATCH], f32, tag=f"{sname}_pool")
                res[sname] = dict(dst=dst_sb, w=ws, b=bs, pool=pool_sb)

            _gq = [0]

            def emit_layer(sname, sd, layer, tab_name, c):
                import os as _os
                _cut = int(_os.environ.get("KLAYERCUT", "9"))
                cd = sd["cores"][c]
                WIN, NT = sd["WIN"], sd["NT"]
                r_ = res[sname]
                subs = cd["subs"]
                fo = sd["w"][layer].shape[1]
                ntr = WIN // 125
                sub_ptr = 0
                slot_base = 0
                for t in range(NT):
                    nsub_t = cd["tiles_nsub"][t]
                    slots_t = cd["tiles_slots"][t]
                    if nsub_t == 0 or _cut < 1:
                        continue
                    gt = gpool.tile([P, sd["CTILE"], 64], f32, tag="G")
                    idxt = pool.tile([P, sd["CTILE"] * 8], i16, tag="idxt")
                    nc.sync.dma_start(
                        out=idxt[:, :slots_t // 16],
                        in_=tens[f"{sname}_idx"][:, slot_base // 16:
                                                 (slot_base + slots_t) // 16])
                    for (ct, rr, o0, o1) in cd["calls"]:
                        if ct != t:
                            continue
                        num = o1 - o0
                        g0 = (o0 - slot_base) // 128
                        reg = nc.gpsimd.alloc_register(
                            f"gn_{sname}_{layer}_{c}_{t}_{rr}")
                        nc.gpsimd.reg_mov(reg, num)
                        nc.gpsimd.dma_gather(
                            out_ap=gt[:, g0:g0 + num // 128, :],
                            in_ap=tens[tab_name][
                                rr * sd["RANGE_SZ"]:(rr + 1) * sd["RANGE_SZ"], :],
                            idxs_ap=idxt[:, (o0 - slot_base) // 16:
                                         (o1 - slot_base) // 16],
                            num_idxs=num, num_idxs_reg=reg, elem_size=64,
                            single_packet=False,
                            queue_num=_gq[0] % 4,
                        )
                        _gq[0] += 1
                        nc.gpsimd.free_register(reg)
                    if _cut < 2:
                        sub_ptr += nsub_t
                        slot_base += slots_t
                        continue
                    wins = {}
                    nwchunks = (nsub_t + WCHUNK - 1) // WCHUNK
                    wts = []
                    for k in range(nwchunks):
                        nk = min(WCHUNK, nsub_t - k * WCHUNK)
                        wt = wpool.tile([P, WCHUNK, SPAN], f32, tag="W")
                        j0 = sub_ptr + k * WCHUNK
                        nc.vector.tensor_tensor(
                            out=wt[:, :nk, :],
                            in0=iota_t[:, None, :].to_broadcast([P, nk, SPAN]),
                            in1=r_["dst"][:, j0:j0 + nk, None].to_broadcast(
                                [P, nk, SPAN]),
                            op=AOP.is_equal,
                        )
                        wts.append(wt)
                    for j in range(nsub_t if _cut >= 3 else 0):
                        sj = sub_ptr + j
                        _t, w_, off, width = subs[sj]
                        if w_ not in wins:
                            wins[w_] = psum_agg.tile([64, WIN], f32,
                                                     tag="aggp", name="aggp")
                            # zero-init the whole window bank so later
                            # accumulates have uniform has_written state
                            nc.tensor.matmul(
                                wins[w_][:], ident[:, :64], zero_t[:, :WIN],
                                start=True, stop=False, skip_group_check=True)
                        nc.tensor.matmul(
                            wins[w_][:, off:off + width],
                            gt[:, j, :],
                            wts[j // WCHUNK][:, j % WCHUNK, :width],
                            start=False,
                            stop=(sj in cd["stops"]),
                            skip_group_check=True,
                        )
                    sub_ptr += nsub_t
                    slot_base += slots_t
                    if _cut < 4:
                        continue
                    for w_, ps in wins.items():
                        dvt = pool.tile([64, WIN], f32, tag="dvt")
                        nc.sync.dma_start(
                            out=dvt[:],
                            in_=tens[f"{sname}_dinv"][:, w_ * WIN:
                                                      (w_ + 1) * WIN])
                        z = pool.tile([64, WIN], f32, tag="z")
                        nc.vector.tensor_tensor(
                            out=z[:], in0=ps[:], in1=dvt[:], op=AOP.mult)
                        yp = psum_y.tile([128, WIN], f32, tag="y")
                        nc.tensor.matmul(yp[:fo, :], r_["w"][layer][:], z[:],
                                         start=True, stop=True)
                        xt = pool.tile([128, WIN], f32, tag="x")
                        nc.scalar.activation(xt[:fo, :], yp[:fo, :], AFT.Relu,
                                             bias=r_["b"][layer][:])
                        if layer < 2 and _cut < 5:
                            continue
                        if layer < 2:
                            xp = pool.tile([64, WIN], f32, tag="xp")
                            nc.vector.tensor_tensor(
                                out=xp[:], in0=xt[:64, :], in1=dvt[:],
                                op=AOP.mult)
                            pt = psum_t.tile([125, ntr, 64], f32, tag="tr")
                            for k in range(ntr):
                                nc.tensor.transpose(
                                    pt[:, k, :], xp[:, k * 125:(k + 1) * 125],
                                    ident[:64, :64])
                            xr = pool.tile([125, ntr, 64], f32, tag="xr")
                            nc.scalar.copy(out=xr[:], in_=pt[:])
                            dst_ap = tens[f"{sname}_ag"][
                                w_ * WIN:(w_ + 1) * WIN, :].rearrange(
                                    "(t p) f -> p t f", t=ntr)
                            nc.sync.dma_start(out=dst_ap, in_=xr[:])
                        else:
                            for (pw, st, ln, col, _addto) in cd["pieces"]:
                                if pw != w_:
                                    continue
                                nc.vector.tensor_reduce(
                                    out=r_["pool"][:, col:col + 1],
                                    in_=xt[:, st:st + ln],
                                    axis=mybir.AxisListType.X,
                                    op=AOP.add)

            # ---------------- main flow
            for sname in sides:
                for c in range(NCORES):
                    with tc.If(pid == c):
                        nc.vector.memset(res[sname]["pool"][:], 0.0)

            import os
            max_ph = int(os.environ.get("KMAXPH", "99"))
            nph = 0
            # interleave p/m layers so each side's AllGather overlaps the
            # other side's gather+matmul work
            for layer in range(3):
                for sname, sd in sides.items():
                    tabs = [f"{sname}_tab1", f"{sname}_tab2", f"{sname}_tab3"]
                    nph += 1
                    if nph > max_ph:
                        continue
                    for c in range(NCORES):
                        with tc.If(pid == c):
                            emit_layer(sname, sd, layer, tabs[layer], c)
                    if layer < 2:
                        nc.gpsimd.collective_compute(
                            "AllGather", AOP.bypass, replica_groups=rg,
                            ins=[tens[f"{sname}_ag"][:]],
                            outs=[tens[tabs[layer + 1]][:]],
                        )
            for sname, sd in sides.items():
                for c in range(NCORES):
                    cd = sd["cores"][c]
                    adds = [(col, g) for (_w, _s, _l, col, g) in cd["pieces"]
                            if g >= 0]
                    if adds:
                        with tc.If(pid == c):
                            for col, g in adds:
                                nc.vector.tensor_tensor(
                                    out=res[sname]["pool"][:, g:g + 1],
                                    in0=res[sname]["pool"][:, g:g + 1],
                                    in1=res[sname]["pool"][:, col:col + 1],
                                    op=AOP.add)

            skips = set(os.environ.get("KSKIP", "").split(","))
            names = list(sides.keys())
            if "ccdma" not in skips:
                for i, sname in enumerate(names):
                    nc.sync.dma_start(out=tens["cc_in"][:, i * G:(i + 1) * G],
                                      in_=res[sname]["pool"][:, :G])
            if "ar" not in skips:
                nc.gpsimd.collective_compute(
                    "AllReduce", AOP.add, replica_groups=rg,
                    ins=[tens["cc_in"][:]], outs=[tens["cc_out"][:]])

            # MLP (replicated on all cores)
            hsum = pool.tile([P, 2 * G], f32, tag="hsum")
            if "mlp" in skips:
                nc.vector.memset(hsum[:], 0.0)
            else:
                nc.gpsimd.dma_start(out=hsum[:], in_=tens["cc_out"][:])
            cinv_t = pool.tile([P, 2 * G], f32, tag="cinvt")
            nc.sync.dma_start(out=cinv_t[:], in_=tens["cinv"][:])
            hmean = pool.tile([P, 2 * G], f32, tag="hmean")
            nc.vector.tensor_tensor(out=hmean[:], in0=hsum[:], in1=cinv_t[:],
                                    op=AOP.mult)

            fw1 = pool.tile([P, 2, 128], f32, tag="fw1")
            nc.sync.dma_start(
                out=fw1[:],
                in_=tens["f_w1"][:].rearrange("(a p) o -> p a o", a=2))
            fb1 = pool.tile([128, 1], f32, tag="fb1")
            nc.sync.dma_start(out=fb1[:], in_=tens["f_b1"][:])
            y1 = psum_y.tile([128, G], f32, tag="y", name="mlp1")
            nc.tensor.matmul(y1[:], fw1[:, 0, :], hmean[:, 0:G],
                             start=True, stop=False, skip_group_check=True)
            nc.tensor.matmul(y1[:], fw1[:, 1, :], hmean[:, G:2 * G],
                             start=False, stop=True, skip_group_check=True)
            h1 = pool.tile([128, G], f32, tag="h1")
            nc.scalar.activation(h1[:], y1[:], AFT.Relu, bias=fb1[:])

            fw2 = pool.tile([128, 64], f32, tag="fw2")
            nc.sync.dma_start(out=fw2[:], in_=tens["f_w2"][:])
            fb2 = pool.tile([64, 1], f32, tag="fb2")
            nc.sync.dma_start(out=fb2[:], in_=tens["f_b2"][:])
            y2 = psum_y.tile([64, G], f32, tag="y", name="mlp2")
            nc.tensor.matmul(y2[:], fw2[:], h1[:], start=True, stop=True)
            h2 = pool.tile([64, G], f32, tag="h2")
            nc.scalar.activation(h2[:], y2[:], AFT.Relu, bias=fb2[:])

            fw3 = pool.tile([64, 32], f32, tag="fw3")
            nc.sync.dma_start(out=fw3[:], in_=tens["f_w3"][:])
            fb3 = pool.tile([32, 1], f32, tag="fb3")
            nc.sync.dma_start(out=fb3[:], in_=tens["f_b3"][:])
            y3 = psum_y.tile([32, G], f32, tag="y", name="mlp3")
            nc.tensor.matmul(y3[:], fw3[:], h2[:], start=True, stop=True)
            h3 = pool.tile([32, G], f32, tag="h3")
            nc.scalar.activation(h3[:], y3[:], AFT.Relu, bias=fb3[:])

            fw4 = pool.tile([32, 1], f32, tag="fw4")
            nc.sync.dma_start(out=fw4[:], in_=tens["f_w4"][:])
            y4 = psum_y.tile([1, G], f32, tag="y", name="mlp4")
            nc.tensor.matmul(y4[:], fw4[:], h3[:], start=True, stop=True)
            yout = pool.tile([1, G], f32, tag="yout")
            nc.vector.tensor_scalar_add(yout[:], y4[:], float(f_b4))
            nc.sync.dma_start(out=out_t[:].rearrange("g o -> o g"),
                              in_=yout[:])

    return nc


# ---------------------------------------------------------------- entry

def _prepare(inputs, win_p=500, win_m=250, range_sz=25000):
    d = {k: np.asarray(v) for k, v in inputs.items()}

    def wd(pre):
        return dict(
            w1=d[f"{pre}_w1"], b1=d[f"{pre}_b1"], w2=d[f"{pre}_w2"],
            b2=d[f"{pre}_b2"], w3=d[f"{pre}_w3"], b3=d[f"{pre}_b3"],
            bn1_g=d[f"{pre}_bn1_g"], bn1_b=d[f"{pre}_bn1_b"],
            bn1_m=d[f"{pre}_bn1_m"], bn1_v=d[f"{pre}_bn1_v"],
            bn2_g=d[f"{pre}_bn2_g"], bn2_b=d[f"{pre}_bn2_b"],
            bn2_m=d[f"{pre}_bn2_m"], bn2_v=d[f"{pre}_bn2_v"],
        )

    sides = {
        "p": _prep_side(d["protein_x"], d["protein_edge"], d["protein_batch"],
                        d["protein_x"].shape[0], d["protein_x"].shape[1],
                        win_p, range_sz, wd("p")),
        "m": _prep_side(d["mol_x"], d["mol_edge"], d["mol_batch"],
                        d["mol_x"].shape[0], d["mol_x"].shape[1],
                        win_m, range_sz, wd("m")),
    }

    nc = _build(sides, float(np.asarray(d["f_b4"]).reshape(-1)[0]))
    from concourse.library_overlay import lower_extended_insts
    lower_extended_insts(nc)
    _split_multi_waits(nc)

    iota = np.tile(np.arange(SPAN, dtype=np.float32), (P, 1))
    ident = np.eye(P, dtype=np.float32)
    cinv = np.tile(np.concatenate([sides["p"]["cinv"], sides["m"]["cinv"]])[None, :],
                   (P, 1)).astype(np.float32)

    in_maps = []
    for c in range(NCORES):
        m = {
            "iota": iota, "identity": ident, "cinv": cinv,
            "f_w1": d["f_w1"].astype(np.float32),
            "f_b1": d["f_b1"].astype(np.float32).reshape(-1, 1),
            "f_w2": d["f_w2"].astype(np.float32),
            "f_b2": d["f_b2"].astype(np.float32).reshape(-1, 1),
            "f_w3": d["f_w3"].astype(np.float32),
            "f_b3": d["f_b3"].astype(np.float32).reshape(-1, 1),
            "f_w4": d["f_w4"].astype(np.float32),
        }
        for sname, sd in sides.items():
            m[f"{sname}_tab1"] = sd["tab1"]
            m[f"{sname}_idx"] = sd["idx_w"][c]
            m[f"{sname}_dstrel"] = sd["dst_w"][c]
            m[f"{sname}_dinv"] = sd["dinv_rep"][c]
            for l in range(3):
                m[f"{sname}_w{l}"] = sd["w"][l]
                m[f"{sname}_b{l}"] = sd["b"][l].reshape(-1, 1)
        in_maps.append(m)

    return nc, in_maps


def _run(inputs, win_p=500, win_m=250, range_sz=25000, trace=False):
    from concourse.bass_utils import run_bass_kernel_spmd
    nc, in_maps = _prepare(inputs, win_p, win_m, range_sz)
    res = run_bass_kernel_spmd(nc, in_maps, core_ids=list(range(NCORES)),
                               trace=trace)
    return res.results[0]["out"].reshape(G, 1).astype(np.float32), res


def kernel(**inputs):
    out, _ = _run(inputs)
    return out



# revision 21
# speedup vs baseline: 1.0527x; 1.0527x over previous
"""Trainium2 Bass kernel for BindingAffinityModel (2x 3-layer GCN + MLP).

Strategy (8 NeuronCores, one SPMD program with per-core If branches):
  - dst-range shard nodes/edges per core; each core gathers neighbor rows
    from a replicated full node table in HBM via SWDGE dma_gather.
  - segment-sum via one-hot matmuls on TensorE accumulating in PSUM
    (per-element has_written semantics handle overlapping column slices).
  - GCN algebra reordering: aggregate in input space (Ahat@x)@W', with bn
    folded into W'/b', and D^-1/2 applied as a table pre-scale (src side)
    plus a per-column post-scale (dst side). Self loops are (d,d) slots.
  - between layers: AllGather of the per-core [Nc,64] shard into the full
    table; final mean-pool partials AllReduce'd; small MLP replicated.
All graph-dependent structure (edges, degrees, batch segments) is
preprocessed on host into index/metadata arrays; per-core instruction
streams are fully specialized inside If(partition_id==c) blocks.

Perf: gathers dominate device time. They run on 4 SWDGE queues
(round-robin per call; each queue ~ one DMA engine's bus share), with
edges sorted by src within each 128-slot block so descriptors walk
ascending HBM addresses (the dst one-hot absorbs the permutation).
Protein/mol phases are interleaved (p1,AGp,m1,AGm,p2,...) so each side's
AllGather and all PE/DVE work hide under the other side's gathers.
single_packet must stay False: True wedges the device.
"""
import sys
import numpy as np

sys.path.insert(0, "/opt/trn_rl_repo")

P = 128
SPAN = 48
WCHUNK = 16
BN_EPS = 1e-5
NCORES = 8
G = 256
SCRATCH = 40  # extra pool columns for multi-piece graphs

_wsplit_ctr = [0]


def _split_multi_waits(nc):
    """Workaround: this walrus build rejects any instruction carrying >1
    sync wait ("Too many sync wait commands"). Move extra waits onto NoOp
    carriers inserted just before the instruction on the same engine."""
    import concourse.mybir as mybir
    nsplit = 0
    for f in nc.m.functions:
        for bb in f.blocks:
            il = bb.instructions
            i = 0
            while i < len(il):
                inst = il[i]
                si = inst.sync_info
                if si is not None and si.on_wait and len(si.on_wait) > 1:
                    import os as _os
                    if _os.environ.get("KSPLITLOG"):
                        print("SPLIT:", inst.name, type(inst).__name__,
                              inst.engine, len(si.on_wait),
                              [w.ant_name for w in si.on_wait])
                    waits = list(si.on_wait)
                    si.on_wait = waits[-1:]
                    for w in waits[:-1]:
                        _wsplit_ctr[0] += 1
                        nop = mybir.InstNoOp(
                            name=f"Wsplit-{_wsplit_ctr[0]}",
                            engine=inst.engine, ins=[], outs=[])
                        nop.sync_info = mybir.SyncInfo(on_wait=[w], on_update=[])
                        il.insert(i, nop)
                        i += 1
                        nsplit += 1
                i += 1
    return nsplit


# ---------------------------------------------------------------- host prep

def _fold_bn(w, b, g, bb, m, v):
    s = (np.asarray(g, np.float64) / np.sqrt(np.asarray(v, np.float64) + BN_EPS))
    wf = (np.asarray(w, np.float64) * s[None, :]).astype(np.float32)
    bf = ((np.asarray(b, np.float64) - np.asarray(m, np.float64)) * s
          + np.asarray(bb, np.float64)).astype(np.float32)
    return wf, bf


def _prep_side(x, edge, batch, N, Fin, WIN, RANGE_SZ, weights):
    """Preprocess one graph side. Returns config + per-core arrays/metadata."""
    src = np.asarray(edge[0], dtype=np.int64)
    dst = np.asarray(edge[1], dtype=np.int64)
    batch = np.asarray(batch, dtype=np.int64)
    x = np.asarray(x, dtype=np.float32)

    NCs = N // NCORES
    NW = NCs // WIN          # windows per core
    WPT = 1                  # windows per gather-tile
    NT = (NW + WPT - 1) // WPT
    R = (N + RANGE_SZ - 1) // RANGE_SZ

    deg = np.bincount(dst, minlength=N).astype(np.float64) + 1.0
    dinv = (1.0 / np.sqrt(deg)).astype(np.float32)

    tab1 = np.zeros((N, 64), dtype=np.float32)
    tab1[:, :Fin] = x * dinv[:, None]

    w1, b1 = _fold_bn(weights["w1"], weights["b1"], weights["bn1_g"],
                      weights["bn1_b"], weights["bn1_m"], weights["bn1_v"])
    w1p = np.zeros((64, 64), dtype=np.float32)
    w1p[:Fin] = w1
    w2, b2 = _fold_bn(weights["w2"], weights["b2"], weights["bn2_g"],
                      weights["bn2_b"], weights["bn2_m"], weights["bn2_v"])
    w3 = np.asarray(weights["w3"], dtype=np.float32)
    b3 = np.asarray(weights["b3"], dtype=np.float32)

    cores = []
    for c in range(NCORES):
        lo, hi = c * NCs, (c + 1) * NCs
        sel = (dst >= lo) & (dst < hi)
        es = np.concatenate([src[sel], np.arange(lo, hi, dtype=np.int64)])
        ed = np.concatenate([dst[sel], np.arange(lo, hi, dtype=np.int64)])
        dl = ed - lo
        wv = dl // WIN
        tv = wv // WPT
        rv = es // RANGE_SZ
        order = np.lexsort((es, dl, rv, tv))
        es, dl, rv, tv = es[order], dl[order], rv[order], tv[order]
        wv = dl // WIN

        # segment boundaries at (tile, range, window) change
        key = (tv * R + rv) * NW + wv
        cuts = np.flatnonzero(np.diff(key)) + 1
        seg_bounds = np.concatenate([[0], cuts, [len(key)]])

        idx_slots, dst_slots = [], []
        subs = []            # (tile, window, off, width)
        calls = []           # (tile, r, slot_out0, slot_out1)
        tiles_nsub = [0] * NT
        tiles_slots = [0] * NT
        cur_call = None
        out_pos = 0

        for si in range(len(seg_bounds) - 1):
            s0, s1 = int(seg_bounds[si]), int(seg_bounds[si + 1])
            t, r, w = int(tv[s0]), int(rv[s0]), int(wv[s0])
            if cur_call is None or cur_call[0] != t or cur_call[1] != r:
                if cur_call is not None:
                    calls.append((cur_call[0], cur_call[1], cur_call[2], out_pos))
                cur_call = (t, r, out_pos)
            seg_dl = dl[s0:s1]
            seg_es = es[s0:s1]
            win_base = w * WIN
            p = s0
            while p < s1:
                sub_base = int(dl[p])
                limit = sub_base + SPAN
                q = min(p + 128, s1)
                over = np.flatnonzero(seg_dl[p - s0:q - s0] >= limit)
                if len(over):
                    q = p + int(over[0])
                n = q - p
                # pad with -1: non-transpose dma_gather skips negative
                # indices (and the dst one-hot zero-masks these slots
                # anyway, so even a fetched row contributes nothing)
                idx_blk = np.full(128, -1, dtype=np.int16)
                dst_blk = np.full(128, -1.0, dtype=np.float32)
                # ascending-src order within the block: gather descriptors
                # then walk increasing HBM addresses (dst one-hot handles
                # the permutation)
                bs = seg_es[p - s0:q - s0]
                bd = seg_dl[p - s0:q - s0]
                perm = np.argsort(bs, kind="stable")
                idx_blk[:n] = (bs[perm] - r * RANGE_SZ).astype(np.int16)
                dst_blk[:n] = (bd[perm] - sub_base).astype(np.float32)
                idx_slots.append(idx_blk)
                dst_slots.append(dst_blk)
                off = sub_base - win_base
                width = min(SPAN, WIN - off)
                assert off + width <= WIN and seg_dl[q - s0 - 1] - sub_base < width
                subs.append((t, w, off, width))
                tiles_nsub[t] += 1
                tiles_slots[t] += 128
                out_pos += 128
                p = q
        if cur_call is not None:
            calls.append((cur_call[0], cur_call[1], cur_call[2], out_pos))

        idx_arr = np.concatenate(idx_slots) if idx_slots else np.zeros(0, np.int16)
        dst_arr = (np.concatenate(dst_slots) if dst_slots
                   else np.zeros(0, np.float32))

        first_seen, last_idx = {}, {}
        for j, (t, w, off, width) in enumerate(subs):
            first_seen.setdefault((t, w), j)
            last_idx[(t, w)] = j
        starts = set(first_seen.values())
        stops = set(last_idx.values())

        # pooling pieces: (window, start_in_window, length, col, add_to_g)
        pieces = []
        scratch_used = 0
        counts_l = np.bincount(batch, minlength=G)
        cum = np.concatenate([[0], np.cumsum(counts_l)])
        for g in range(G):
            glo, ghi = max(int(cum[g]), lo), min(int(cum[g + 1]), hi)
            if glo >= ghi:
                continue
            first = True
            a = glo
            while a < ghi:
                w = (a - lo) // WIN
                b_ = min(ghi, lo + (w + 1) * WIN)
                if first:
                    col, addto = g, -1
                else:
                    col, addto = G + scratch_used, g
                    scratch_used += 1
                pieces.append((w, (a - lo) - w * WIN, b_ - a, col, addto))
                first = False
                a = b_
        assert scratch_used <= SCRATCH, scratch_used

        cores.append(dict(
            idx=idx_arr, dstrel=dst_arr, subs=subs, calls=calls,
            starts=starts, stops=stops, tiles_nsub=tiles_nsub,
            tiles_slots=tiles_slots, pieces=pieces,
            dinv_shard=dinv[lo:hi],
        ))

    S = max(len(cc["idx"]) for cc in cores)
    S = max(((S + 2047) // 2048) * 2048, 2048)
    NSUB = S // 128
    CTILE = max(max((s // 128 for s in cc["tiles_slots"]), default=1)
                for cc in cores)
    idx_w = np.zeros((NCORES, P, S // 16), dtype=np.int16)
    dst_w = np.full((NCORES, P, NSUB), -1.0, dtype=np.float32)
    dinv_rep = np.zeros((NCORES, 64, NCs), dtype=np.float32)
    for c, cc in enumerate(cores):
        n = len(cc["idx"])
        idx_pad = np.concatenate([cc["idx"], np.zeros(S - n, np.int16)])
        dst_pad = np.concatenate([cc["dstrel"], np.full(S - n, -1.0, np.float32)])
        idx_w[c] = np.tile(idx_pad.reshape(-1, 16).T, (8, 1))
        dst_w[c] = dst_pad.reshape(-1, 128).T
        dinv_rep[c] = np.tile(cc["dinv_shard"][None, :], (64, 1))

    counts = np.bincount(batch, minlength=G).astype(np.float32)
    cinv = (1.0 / np.maximum(counts, 1.0)).astype(np.float32)

    return dict(
        N=N, NCs=NCs, WIN=WIN, NW=NW, WPT=WPT, NT=NT, R=R, RANGE_SZ=RANGE_SZ,
        S=S, NSUB=NSUB, CTILE=CTILE,
        tab1=tab1, w=[w1p, w2, w3], b=[b1, b2, b3],
        idx_w=idx_w, dst_w=dst_w, dinv_rep=dinv_rep, cinv=cinv,
        cores=cores,
    )


# ---------------------------------------------------------------- builder

def _build(sides, f_b4):
    import concourse.bass as bass
    import concourse.mybir as mybir
    import concourse.tile as tile
    from concourse import library_config

    f32 = mybir.dt.float32
    i16 = mybir.dt.int16
    AOP = mybir.AluOpType
    AFT = mybir.ActivationFunctionType

    nc = bass.Bass("TRN2", num_devices=NCORES, num_swdge_queues=4)

    tens = {}

    def inp(name, shape, dtype=f32):
        tens[name] = nc.dram_tensor(name, list(shape), dtype,
                                    kind="ExternalInput")
        return tens[name]

    for sname, sd in sides.items():
        inp(f"{sname}_tab1", (sd["N"], 64))
        inp(f"{sname}_idx", (P, sd["S"] // 16), i16)
        inp(f"{sname}_dstrel", (P, sd["NSUB"]))
        inp(f"{sname}_dinv", (64, sd["NCs"]))
        for l in range(3):
            fo = sd["w"][l].shape[1]
            inp(f"{sname}_w{l}", (64, fo))
            inp(f"{sname}_b{l}", (fo, 1))
    inp("iota", (P, SPAN))
    inp("identity", (P, P))
    inp("cinv", (P, 2 * G))
    inp("f_w1", (256, 128)); inp("f_b1", (128, 1))
    inp("f_w2", (128, 64)); inp("f_b2", (64, 1))
    inp("f_w3", (64, 32)); inp("f_b3", (32, 1))
    inp("f_w4", (32, 1))
    out_t = nc.dram_tensor("out", [G, 1], f32, kind="ExternalOutput")

    for sname, sd in sides.items():
        tens[f"{sname}_ag"] = nc.dram_tensor(
            f"{sname}_ag", [sd["NCs"], 64], f32)
        tens[f"{sname}_tab2"] = nc.dram_tensor(
            f"{sname}_tab2", [sd["N"], 64], f32, addr_space="Shared")
        tens[f"{sname}_tab3"] = nc.dram_tensor(
            f"{sname}_tab3", [sd["N"], 64], f32, addr_space="Shared")
    tens["cc_in"] = nc.dram_tensor("cc_in", [P, 2 * G], f32)
    tens["cc_out"] = nc.dram_tensor("cc_out", [P, 2 * G], f32,
                                    addr_space="Shared")

    nc.gpsimd.load_library(library_config.attnmlp)

    rg = [list(range(NCORES))]

    with tile.TileContext(nc) as tc:
        pid = nc.partition_id()
        with tc.tile_pool(name="const", bufs=1) as cpool, \
             tc.tile_pool(name="sbuf", bufs=3) as pool, \
             tc.tile_pool(name="gpool", bufs=3) as gpool, \
             tc.tile_pool(name="wpool", bufs=3) as wpool, \
             tc.tile_pool(name="agg", bufs=4, space="PSUM") as psum_agg, \
             tc.tile_pool(name="py", bufs=2, space="PSUM") as psum_y, \
             tc.tile_pool(name="pt", bufs=2, space="PSUM") as psum_t:

            iota_t = cpool.tile([P, SPAN], f32)
            nc.sync.dma_start(out=iota_t[:], in_=tens["iota"][:])
            ident = cpool.tile([P, P], f32)
            nc.sync.dma_start(out=ident[:], in_=tens["identity"][:])
            zero_t = cpool.tile([P, 512], f32)
            nc.vector.memset(zero_t[:], 0.0)

            res = {}
            for sname, sd in sides.items():
                dst_sb = cpool.tile([P, sd["NSUB"]], f32, tag=f"{sname}_dst")
                nc.sync.dma_start(out=dst_sb[:], in_=tens[f"{sname}_dstrel"][:])
                ws, bs = [], []
                for l in range(3):
                    fo = sd["w"][l].shape[1]
                    wt = cpool.tile([64, fo], f32, tag=f"{sname}_w{l}")
                    nc.sync.dma_start(out=wt[:], in_=tens[f"{sname}_w{l}"][:])
                    bt = cpool.tile([fo, 1], f32, tag=f"{sname}_b{l}")
                    nc.sync.dma_start(out=bt[:], in_=tens[f"{sname}_b{l}"][:])
                    ws.append(wt)
                    bs.append(bt)
                pool_sb = cpool.tile([P, G + SCRATCH], f32, tag=f"{sname}_pool")
                res[sname] = dict(dst=dst_sb, w=ws, b=bs, pool=pool_sb)

            _gq = [0]

            def emit_layer(sname, sd, layer, tab_name, c):
                import os as _os
                _cut = int(_os.environ.get("KLAYERCUT", "9"))
                cd = sd["cores"][c]
                WIN, NT = sd["WIN"], sd["NT"]
                r_ = res[sname]
                subs = cd["subs"]
                fo = sd["w"][layer].shape[1]
                ntr = WIN // 125
                sub_ptr = 0
                slot_base = 0
                for t in range(NT):
                    nsub_t = cd["tiles_nsub"][t]
                    slots_t = cd["tiles_slots"][t]
                    if nsub_t == 0 or _cut < 1:
                        continue
                    gt = gpool.tile([P, sd["CTILE"], 64], f32, tag="G")
                    idxt = pool.tile([P, sd["CTILE"] * 8], i16, tag="idxt")
                    nc.sync.dma_start(
                        out=idxt[:, :slots_t // 16],
                        in_=tens[f"{sname}_idx"][:, slot_base // 16:
                                                 (slot_base + slots_t) // 16])
                    for (ct, rr, o0, o1) in cd["calls"]:
                        if ct != t:
                            continue
                        num = o1 - o0
                        g0 = (o0 - slot_base) // 128
                        reg = nc.gpsimd.alloc_register(
                            f"gn_{sname}_{layer}_{c}_{t}_{rr}")
                        nc.gpsimd.reg_mov(reg, num)
                        nc.gpsimd.dma_gather(
                            out_ap=gt[:, g0:g0 + num // 128, :],
                            in_ap=tens[tab_name][
                                rr * sd["RANGE_SZ"]:(rr + 1) * sd["RANGE_SZ"], :],
                            idxs_ap=idxt[:, (o0 - slot_base) // 16:
                                         (o1 - slot_base) // 16],
                            num_idxs=num, num_idxs_reg=reg, elem_size=64,
                            single_packet=False,
                            queue_num=_gq[0] % 4,
                        )
                        _gq[0] += 1
                        nc.gpsimd.free_register(reg)
                    if _cut < 2:
                        sub_ptr += nsub_t
                        slot_base += slots_t
                        continue
                    wins = {}
                    nwchunks = (nsub_t + WCHUNK - 1) // WCHUNK
                    wts = []
                    for k in range(nwchunks):
                        nk = min(WCHUNK, nsub_t - k * WCHUNK)
                        wt = wpool.tile([P, WCHUNK, SPAN], f32, tag="W")
                        j0 = sub_ptr + k * WCHUNK
                        nc.vector.tensor_tensor(
                            out=wt[:, :nk, :],
                            in0=iota_t[:, None, :].to_broadcast([P, nk, SPAN]),
                            in1=r_["dst"][:, j0:j0 + nk, None].to_broadcast(
                                [P, nk, SPAN]),
                            op=AOP.is_equal,
                        )
                        wts.append(wt)
                    for j in range(nsub_t if _cut >= 3 else 0):
                        sj = sub_ptr + j
                        _t, w_, off, width = subs[sj]
                        if w_ not in wins:
                            wins[w_] = psum_agg.tile([64, WIN], f32,
                                                     tag="aggp", name="aggp")
                            # zero-init the whole window bank so later
                            # accumulates have uniform has_written state
                            nc.tensor.matmul(
                                wins[w_][:], ident[:, :64], zero_t[:, :WIN],
                                start=True, stop=False, skip_group_check=True)
                        nc.tensor.matmul(
                            wins[w_][:, off:off + width],
                            gt[:, j, :],
                            wts[j // WCHUNK][:, j % WCHUNK, :width],
                            start=False,
                            stop=(sj in cd["stops"]),
                            skip_group_check=True,
                        )
                    sub_ptr += nsub_t
                    slot_base += slots_t
                    if _cut < 4:
                        continue
                    for w_, ps in wins.items():
                        dvt = pool.tile([64, WIN], f32, tag="dvt")
                        nc.sync.dma_start(
                            out=dvt[:],
                            in_=tens[f"{sname}_dinv"][:, w_ * WIN:
                                                      (w_ + 1) * WIN])
                        z = pool.tile([64, WIN], f32, tag="z")
                        nc.vector.tensor_tensor(
                            out=z[:], in0=ps[:], in1=dvt[:], op=AOP.mult)
                        yp = psum_y.tile([128, WIN], f32, tag="y")
                        nc.tensor.matmul(yp[:fo, :], r_["w"][layer][:], z[:],
                                         start=True, stop=True)
                        xt = pool.tile([128, WIN], f32, tag="x")
                        nc.scalar.activation(xt[:fo, :], yp[:fo, :], AFT.Relu,
                                             bias=r_["b"][layer][:])
                        if layer < 2 and _cut < 5:
                            continue
                        if layer < 2:
                            xp = pool.tile([64, WIN], f32, tag="xp")
                            nc.vector.tensor_tensor(
                                out=xp[:], in0=xt[:64, :], in1=dvt[:],
                                op=AOP.mult)
                            pt = psum_t.tile([125, ntr, 64], f32, tag="tr")
                            for k in range(ntr):
                                nc.tensor.transpose(
                                    pt[:, k, :], xp[:, k * 125:(k + 1) * 125],
                                    ident[:64, :64])
                            xr = pool.tile([125, ntr, 64], f32, tag="xr")
                            nc.scalar.copy(out=xr[:], in_=pt[:])
                            dst_ap = tens[f"{sname}_ag"][
                                w_ * WIN:(w_ + 1) * WIN, :].rearrange(
                                    "(t p) f -> p t f", t=ntr)
                            nc.sync.dma_start(out=dst_ap, in_=xr[:])
                        else:
                            for (pw, st, ln, col, _addto) in cd["pieces"]:
                                if pw != w_:
                                    continue
                                nc.vector.tensor_reduce(
                                    out=r_["pool"][:, col:col + 1],
                                    in_=xt[:, st:st + ln],
                                    axis=mybir.AxisListType.X,
                                    op=AOP.add)

            # ---------------- main flow
            for sname in sides:
                for c in range(NCORES):
                    with tc.If(pid == c):
                        nc.vector.memset(res[sname]["pool"][:], 0.0)

            import os
            max_ph = int(os.environ.get("KMAXPH", "99"))
            nph = 0
            # interleave p/m layers so each side's AllGather overlaps the
            # other side's gather+matmul work
            for layer in range(3):
                for sname, sd in sides.items():
                    tabs = [f"{sname}_tab1", f"{sname}_tab2", f"{sname}_tab3"]
                    nph += 1
                    if nph > max_ph:
                        continue
                    for c in range(NCORES):
                        with tc.If(pid == c):
                            emit_layer(sname, sd, layer, tabs[layer], c)
                    if layer < 2:
                        nc.gpsimd.collective_compute(
                            "AllGather", AOP.bypass, replica_groups=rg,
                            ins=[tens[f"{sname}_ag"][:]],
                            outs=[tens[tabs[layer + 1]][:]],
                        )
            for sname, sd in sides.items():
                for c in range(NCORES):
                    cd = sd["cores"][c]
                    adds = [(col, g) for (_w, _s, _l, col, g) in cd["pieces"]
                            if g >= 0]
                    if adds:
                        with tc.If(pid == c):
                            for col, g in adds:
                                nc.vector.tensor_tensor(
                                    out=res[sname]["pool"][:, g:g + 1],
                                    in0=res[sname]["pool"][:, g:g + 1],
                                    in1=res[sname]["pool"][:, col:col + 1],
                                    op=AOP.add)

            skips = set(os.environ.get("KSKIP", "").split(","))
            names = list(sides.keys())
            if "ccdma" not in skips:
                for i, sname in enumerate(names):
                    nc.sync.dma_start(out=tens["cc_in"][:, i * G:(i + 1) * G],
                                      in_=res[sname]["pool"][:, :G])
            if "ar" not in skips:
                nc.gpsimd.collective_compute(
                    "AllReduce", AOP.add, replica_groups=rg,
                    ins=[tens["cc_in"][:]], outs=[tens["cc_out"][:]])

            # MLP (replicated on all cores)
            hsum = pool.tile([P, 2 * G], f32, tag="hsum")
            if "mlp" in skips:
                nc.vector.memset(hsum[:], 0.0)
            else:
                nc.gpsimd.dma_start(out=hsum[:], in_=tens["cc_out"][:])
            cinv_t = pool.tile([P, 2 * G], f32, tag="cinvt")
            nc.sync.dma_start(out=cinv_t[:], in_=tens["cinv"][:])
            hmean = pool.tile([P, 2 * G], f32, tag="hmean")
            nc.vector.tensor_tensor(out=hmean[:], in0=hsum[:], in1=cinv_t[:],
                                    op=AOP.mult)

            fw1 = pool.tile([P, 2, 128], f32, tag="fw1")
            nc.sync.dma_start(
                out=fw1[:],
                in_=tens["f_w1"][:].rearrange("(a p) o -> p a o", a=2))
            fb1 = pool.tile([128, 1], f32, tag="fb1")
            nc.sync.dma_start(out=fb1[:], in_=tens["f_b1"][:])
            y1 = psum_y.tile([128, G], f32, tag="y", name="mlp1")
            nc.tensor.matmul(y1[:], fw1[:, 0, :], hmean[:, 0:G],
                             start=True, stop=False, skip_group_check=True)
            nc.tensor.matmul(y1[:], fw1[:, 1, :], hmean[:, G:2 * G],
                             start=False, stop=True, skip_group_check=True)
            h1 = pool.tile([128, G], f32, tag="h1")
            nc.scalar.activation(h1[:], y1[:], AFT.Relu, bias=fb1[:])

            fw2 = pool.tile([128, 64], f32, tag="fw2")
            nc.sync.dma_start(out=fw2[:], in_=tens["f_w2"][:])
            fb2 = pool.tile([64, 1], f32, tag="fb2")
            nc.sync.dma_start(out=fb2[:], in_=tens["f_b2"][:])
            y2 = psum_y.tile([64, G], f32, tag="y", name="mlp2")
            nc.tensor.matmul(y2[:], fw2[:], h1[:], start=True, stop=True)
            h2 = pool.tile([64, G], f32, tag="h2")
            nc.scalar.activation(h2[:], y2[:], AFT.Relu, bias=fb2[:])

            fw3 = pool.tile([64, 32], f32, tag="fw3")
            nc.sync.dma_start(out=fw3[:], in_=tens["f_w3"][:])
            fb3 = pool.tile([32, 1], f32, tag="fb3")
            nc.sync.dma_start(out=fb3[:], in_=tens["f_b3"][:])
            y3 = psum_y.tile([32, G], f32, tag="y", name="mlp3")
            nc.tensor.matmul(y3[:], fw3[:], h2[:], start=True, stop=True)
            h3 = pool.tile([32, G], f32, tag="h3")
            nc.scalar.activation(h3[:], y3[:], AFT.Relu, bias=fb3[:])

            fw4 = pool.tile([32, 1], f32, tag="fw4")
            nc.sync.dma_start(out=fw4[:], in_=tens["f_w4"][:])
            y4 = psum_y.tile([1, G], f32, tag="y", name="mlp4")
            nc.tensor.matmul(y4[:], fw4[:], h3[:], start=True, stop=True)
            yout = pool.tile([1, G], f32, tag="yout")
            nc.vector.tensor_scalar_add(yout[:], y4[:], float(f_b4))
            nc.sync.dma_start(out=out_t[:].rearrange("g o -> o g"),
                              in_=yout[:])

    return nc


# ---------------------------------------------------------------- entry

def _prepare(inputs, win_p=500, win_m=250, range_sz=25000):
    d = {k: np.asarray(v) for k, v in inputs.items()}

    def wd(pre):
        return dict(
            w1=d[f"{pre}_w1"], b1=d[f"{pre}_b1"], w2=d[f"{pre}_w2"],
            b2=d[f"{pre}_b2"], w3=d[f"{pre}_w3"], b3=d[f"{pre}_b3"],
            bn1_g=d[f"{pre}_bn1_g"], bn1_b=d[f"{pre}_bn1_b"],
            bn1_m=d[f"{pre}_bn1_m"], bn1_v=d[f"{pre}_bn1_v"],
            bn2_g=d[f"{pre}_bn2_g"], bn2_b=d[f"{pre}_bn2_b"],
            bn2_m=d[f"{pre}_bn2_m"], bn2_v=d[f"{pre}_bn2_v"],
        )

    sides = {
        "p": _prep_side(d["protein_x"], d["protein_edge"], d["protein_batch"],
                        d["protein_x"].shape[0], d["protein_x"].shape[1],
                        win_p, range_sz, wd("p")),
        "m": _prep_side(d["mol_x"], d["mol_edge"], d["mol_batch"],
                        d["mol_x"].shape[0], d["mol_x"].shape[1],
                        win_m, range_sz, wd("m")),
    }

    nc = _build(sides, float(np.asarray(d["f_b4"]).reshape(-1)[0]))
    from concourse.library_overlay import lower_extended_insts
    lower_extended_insts(nc)
    _split_multi_waits(nc)

    iota = np.tile(np.arange(SPAN, dtype=np.float32), (P, 1))
    ident = np.eye(P, dtype=np.float32)
    cinv = np.tile(np.concatenate([sides["p"]["cinv"], sides["m"]["cinv"]])[None, :],
                   (P, 1)).astype(np.float32)

    in_maps = []
    for c in range(NCORES):
        m = {
            "iota": iota, "identity": ident, "cinv": cinv,
            "f_w1": d["f_w1"].astype(np.float32),
            "f_b1": d["f_b1"].astype(np.float32).reshape(-1, 1),
            "f_w2": d["f_w2"].astype(np.float32),
            "f_b2": d["f_b2"].astype(np.float32).reshape(-1, 1),
            "f_w3": d["f_w3"].astype(np.float32),
            "f_b3": d["f_b3"].astype(np.float32).reshape(-1, 1),
            "f_w4": d["f_w4"].astype(np.float32),
        }
        for sname, sd in sides.items():
            m[f"{sname}_tab1"] = sd["tab1"]
            m[f"{sname}_idx"] = sd["idx_w"][c]
            m[f"{sname}_dstrel"] = sd["dst_w"][c]
            m[f"{sname}_dinv"] = sd["dinv_rep"][c]
            for l in range(3):
                m[f"{sname}_w{l}"] = sd["w"][l]
                m[f"{sname}_b{l}"] = sd["b"][l].reshape(-1, 1)
        in_maps.append(m)

    return nc, in_maps


def _run(inputs, win_p=500, win_m=250, range_sz=25000, trace=False):
    from concourse.bass_utils import run_bass_kernel_spmd
    nc, in_maps = _prepare(inputs, win_p, win_m, range_sz)
    res = run_bass_kernel_spmd(nc, in_maps, core_ids=list(range(NCORES)),
                               trace=trace)
    return res.results[0]["out"].reshape(G, 1).astype(np.float32), res


def kernel(**inputs):
    out, _ = _run(inputs)
    return out

